# revision 24
# baseline (speedup 1.0000x reference)
"""AdaptiveChebConv (K=3) distributed Bass kernel for 8 TRN2 NeuronCores.

Data-parallel over batch: B=16 -> 2 batches per core. adj/Theta replicated.

Per-core algorithm (per local batch b; N=1024, F=O=64, T=12), using the
commutation of the Theta (feature) contraction with the A (node) hops:

  out = relu(W0 + A^T (W1 + A^T W2)),   W_k[n,o,t] = sum_f X[n,f,t] Theta_k[f,o]

The host supplies X^T (xt = x.transpose(0,2,3,1), bf16) so each W_k is a
K=64 matmul with X^T tiles as the stationary operand -- no on-device
transposes at all. All intermediates in (t,o)-major layout; the final
relu-copy de-swizzles to the natural (o,t) output layout.
"""
import sys

if "/opt/trn_rl_repo" not in sys.path:
    sys.path.insert(0, "/opt/trn_rl_repo")

import numpy as np
from contextlib import ExitStack

import concourse.bass as bass
import concourse.tile as tile
from concourse import bacc, mybir
from concourse.bass_utils import run_bass_kernel_spmd

N_CORES = 8
B, N, F, T, K, O = 16, 1024, 64, 12, 3, 64
BL = B // N_CORES          # local batches per core = 2
NT = N // 128              # n-tiles = 8
FT = F * T                 # 768
OT = O * T                 # 768

F32 = mybir.dt.float32
BF16 = mybir.dt.bfloat16

_NC = None


class Ctx:
    pass


def _emit_A(cx, b, load_adj=False):
    """A(b) = adj * attn[b] as 8 per-mt bf16 tiles."""
    nc = cx.nc
    A_t = []
    for mt in range(NT):
        if load_adj:
            adj_t = cx.const_pool.tile(
                [128, 1024], BF16, tag=f"adj{mt}", name=f"adj{mt}"
            )
            da = nc.sync.dma_start(
                adj_t[:], cx.adj_ap[mt * 128:(mt + 1) * 128, :]
            )
            if cx.attn_dep is not None:
                bass._add_dep_helper(
                    da.ins, cx.attn_dep.ins, True, "dma phase order"
                )
            cx.adj_t.append(adj_t)
        attn_s = cx.scr_pool.tile(
            [128, 1024], BF16, tag="attnscr", bufs=3, name=f"at{b}_{mt}"
        )
        d = nc.sync.dma_start(
            attn_s[:], cx.attn_ap[b, mt * 128:(mt + 1) * 128, :]
        )
        if cx.attn_dep is not None:
            bass._add_dep_helper(d.ins, cx.attn_dep.ins, True, "dma phase order")
        cx.last_attn = d
        a = cx.a_pool.tile(
            [128, 1024], BF16, tag=f"A{mt}", bufs=2, name=f"A{b}_{mt}"
        )
        nc.vector.tensor_mul(a[:], attn_s[:], cx.adj_t[mt][:])
        A_t.append(a)
    return A_t


def _emit_theta_mms(cx, pq, b, nt, ch, kk, start, stop):
    """6 accumulating K=64 matmuls: psum[:, ti*64:+64] += T0_slice^T Theta_kk."""
    nc = cx.nc
    for ti in range(6):
        t = ch * 6 + ti
        nc.tensor.matmul(
            pq[:, ti * 64:(ti + 1) * 64],
            cx.T_t[(b, nt)][:, t * 128:(t + 1) * 128],
            cx.theta_t[:, kk * 64:(kk + 1) * 64],
            start=(start and ti == 0),
            stop=(stop and ti == 5),
        )


def _emit_w12(cx, b, W1, W2):
    """W1, W2 = X . Theta_{1,2} in (t,o)-major layout via free=128 matmuls
    against the stacked [Theta1|Theta2] moving operand."""
    nc = cx.nc
    for nt in range(NT):
        pqs = []
        for g in range(3):  # 4 t's per psum tile: cols (tl, k, o)
            pq = cx.zp.tile([128, 512], F32, tag="zp", name="pq")
            for tl in range(4):
                t = 4 * g + tl
                nc.tensor.matmul(
                    pq[:, tl * 128:(tl + 1) * 128],
                    cx.T_t[(b, nt)][:, t * 128:(t + 1) * 128],
                    cx.theta_t[:, 64:192],
                    start=True,
                    stop=True,
                )
            pqs.append(pq)
        for g in range(3):
            pq4 = pqs[g][:].rearrange("p (tl ko) -> p tl ko", ko=128)
            for kk, W in ((0, W1), (1, W2)):
                dst = W[:, nt * FT + 4 * g * 64: nt * FT + (4 * g + 4) * 64]
                nc.vector.tensor_copy(
                    dst.rearrange("p (tl o) -> p tl o", o=64),
                    pq4[:, :, kk * 64:(kk + 1) * 64],
                )


def _emit_hop(cx, b, A_t, rhs_all, kk, dst, addend=None):
    """dst = A^T rhs_all + addend (or inline X.Theta_kk Theta MMs).

    If dst is None this is the output stage: relu-copy to out tiles + DMA.
    """
    nc = cx.nc
    for nt in range(NT):
        o_tile = None
        if dst is None:
            o_tile = cx.out_pool.tile([128, OT], F32, tag="out", name="o_tile")
        pzs = []
        for ch in range(2):
            pzf = cx.zp.tile([128, 512], F32, tag="zp", name="pz")
            pz = pzf[:, 0:384]
            if addend is None:
                _emit_theta_mms(cx, pz, b, nt, ch, kk, True, False)
            pzs.append(pz)
        for ch in range(2):
            pz = pzs[ch]
            for mt in range(NT):
                nc.tensor.matmul(
                    pz,
                    A_t[mt][:, nt * 128:(nt + 1) * 128],
                    rhs_all[:, mt * FT + ch * 384: mt * FT + (ch + 1) * 384],
                    start=(addend is not None and mt == 0),
                    stop=(mt == NT - 1),
                )
            sl = slice(nt * FT + ch * 384, nt * FT + (ch + 1) * 384)
            if dst is not None:
                nc.vector.tensor_add(dst[:, sl], pz, addend[:, sl])
            else:
                d = o_tile[:].rearrange("p (o t) -> p t o", t=T)[
                    :, ch * 6:(ch + 1) * 6, :
                ]
                s = pz.rearrange("p (t o) -> p t o", o=64)
                nc.scalar.activation(d, s, mybir.ActivationFunctionType.Relu)
        if dst is None:
            nc.sync.dma_start(
                cx.out_ap[b, nt * 128:(nt + 1) * 128, :, :].rearrange(
                    "p o t -> p (o t)"
                ),
                o_tile[:],
            )


def _build():
    nc = bacc.Bacc("TRN2", target_bir_lowering=False, debug=False)
    cx = Ctx()
    cx.nc = nc
    cx.xt_ap = nc.dram_tensor("xt", [BL, NT, F, T, 128], BF16, kind="ExternalInput").ap()
    cx.attn_ap = nc.dram_tensor(
        "spatial_attention", [BL, N, N], BF16, kind="ExternalInput"
    ).ap()
    cx.adj_ap = nc.dram_tensor("adj", [N, N], BF16, kind="ExternalInput").ap()
    cx.theta_ap = nc.dram_tensor("Theta", [K, F, O], BF16, kind="ExternalInput").ap()
    cx.out_ap = nc.dram_tensor("out", [BL, N, O, T], F32, kind="ExternalOutput").ap()

    with tile.TileContext(nc) as tc, ExitStack() as ctx:
        cx.a_pool = ctx.enter_context(tc.tile_pool(name="apool", bufs=2))
        cx.w_pool = ctx.enter_context(tc.tile_pool(name="wpool", bufs=2))
        cx.t_pool = ctx.enter_context(tc.tile_pool(name="tpool", bufs=1))
        cx.scr_pool = ctx.enter_context(tc.tile_pool(name="scr", bufs=3))
        cx.out_pool = ctx.enter_context(tc.tile_pool(name="outp", bufs=3))
        cx.const_pool = ctx.enter_context(tc.tile_pool(name="const", bufs=1))
        cx.zp = ctx.enter_context(tc.tile_pool(name="zp", bufs=7, space="PSUM"))

        cx.theta_t = cx.const_pool.tile([64, K * O], BF16, tag="theta")
        nc.sync.dma_start(cx.theta_t[:], cx.theta_ap.rearrange("k f o -> f k o"))
        cx.adj_t = []
        cx.attn_dep = None

        # T0 tiles: host-transposed X^T per (batch, n-tile): [64, T*128]
        cx.T_t = {}
        for b in range(BL):
            for nt in range(NT):
                cx.T_t[(b, nt)] = cx.t_pool.tile(
                    [64, T * 128], BF16, tag=f"T0_{nt}_{b}", name=f"T{b}_{nt}"
                )
        for b in range(BL):
            for nt in range(NT):
                cx.attn_dep = nc.sync.dma_start(
                    cx.T_t[(b, nt)][:],
                    cx.xt_ap[b, nt].rearrange("f t n -> f (t n)"),
                )

        # W1/W2 for both batches early (PE filler during attn/adj DMA).
        W1 = [
            cx.w_pool.tile([128, NT * FT], BF16, tag="W1", name=f"W1_{b}")
            for b in range(BL)
        ]
        W2 = [
            cx.w_pool.tile([128, NT * FT], BF16, tag="W2", name=f"W2_{b}")
            for b in range(BL)
        ]
        for b in range(BL):
            _emit_w12(cx, b, W1[b], W2[b])

        for b in range(BL):
            A_t = _emit_A(cx, b, load_adj=(b == 0))
            if b == 0:
                cx.attn_dep = cx.last_attn  # gate batch-1 attn behind batch-0
            V = cx.w_pool.tile(
                [128, NT * FT], BF16, tag="V", name=f"V_{b}", bufs=1
            )
            _emit_hop(cx, b, A_t, W2[b], 1, V, addend=W1[b])
            _emit_hop(cx, b, A_t, V, 0, None)

    nc.compile()
    return nc


def _make_in_maps(inputs):
    import ml_dtypes

    bf = ml_dtypes.bfloat16
    x = np.asarray(inputs["x"], dtype=np.float32).astype(bf)
    # [B, N, F, T] -> [B, NT, F, T, 128] so each (batch, n-tile) block is
    # one contiguous 192KB DMA
    xt = np.ascontiguousarray(
        x.reshape(B, NT, 128, F, T).transpose(0, 1, 3, 4, 2)
    )
    attn = np.ascontiguousarray(
        np.asarray(inputs["spatial_attention"], dtype=np.float32).astype(bf)
    )
    adj = np.ascontiguousarray(np.asarray(inputs["adj"], dtype=np.float32).astype(bf))
    theta = np.ascontiguousarray(
        np.asarray(inputs["Theta"], dtype=np.float32).astype(bf)
    )

    in_maps = []
    for i in range(N_CORES):
        s = slice(i * BL, (i + 1) * BL)
        in_maps.append(
            {
                "xt": xt[s],
                "spatial_attention": attn[s],
                "adj": adj,
                "Theta": theta,
            }
        )
    return in_maps


def kernel(**inputs):
    global _NC
    if _NC is None:
        _NC = _build()
    nc = _NC
    in_maps = _make_in_maps(inputs)
    res = run_bass_kernel_spmd(nc, in_maps, core_ids=list(range(N_CORES)))
    out = np.concatenate([res.results[i]["out"] for i in range(N_CORES)], axis=0)
    return out


# revision 25
# speedup vs baseline: 1.0338x; 1.0338x over previous
"""AdaptiveChebConv (K=3) distributed Bass kernel for 8 TRN2 NeuronCores.

Data-parallel over batch: B=16 -> 2 batches per core. adj/Theta replicated.

Per-core algorithm (per local batch b; N=1024, F=O=64, T=12), using the
commutation of the Theta (feature) contraction with the A (node) hops:

  out = relu(W0 + A^T (W1 + A^T W2)),   W_k[n,o,t] = sum_f X[n,f,t] Theta_k[f,o]

The host supplies X^T (xt = x.transpose(0,2,3,1), bf16) so each W_k is a
K=64 matmul with X^T tiles as the stationary operand -- no on-device
transposes at all. All intermediates in (t,o)-major layout; the final
relu-copy de-swizzles to the natural (o,t) output layout.
"""
import sys

if "/opt/trn_rl_repo" not in sys.path:
    sys.path.insert(0, "/opt/trn_rl_repo")

import numpy as np
from contextlib import ExitStack

import concourse.bass as bass
import concourse.tile as tile
from concourse import bacc, mybir
from concourse.bass_utils import run_bass_kernel_spmd

N_CORES = 8
B, N, F, T, K, O = 16, 1024, 64, 12, 3, 64
BL = B // N_CORES          # local batches per core = 2
NT = N // 128              # n-tiles = 8
FT = F * T                 # 768
OT = O * T                 # 768

F32 = mybir.dt.float32
BF16 = mybir.dt.bfloat16

_NC = None


class Ctx:
    pass


def _emit_A(cx, b, load_adj=False):
    """A(b) = adj * attn[b] as 8 per-mt bf16 tiles."""
    nc = cx.nc
    A_t = []
    for mt in range(NT):
        if load_adj:
            adj_t = cx.const_pool.tile(
                [128, 1024], BF16, tag=f"adj{mt}", name=f"adj{mt}"
            )
            da = nc.sync.dma_start(
                adj_t[:], cx.adj_ap[mt * 128:(mt + 1) * 128, :]
            )
            if cx.attn_dep is not None:
                bass._add_dep_helper(
                    da.ins, cx.attn_dep.ins, True, "dma phase order"
                )
            cx.adj_t.append(adj_t)
        attn_s = cx.scr_pool.tile(
            [128, 1024], BF16, tag="attnscr", bufs=3, name=f"at{b}_{mt}"
        )
        d = nc.sync.dma_start(
            attn_s[:], cx.attn_ap[b, mt * 128:(mt + 1) * 128, :]
        )
        if cx.attn_dep is not None:
            bass._add_dep_helper(d.ins, cx.attn_dep.ins, True, "dma phase order")
        cx.last_attn = d
        a = cx.a_pool.tile(
            [128, 1024], BF16, tag=f"A{mt}", bufs=2, name=f"A{b}_{mt}"
        )
        nc.vector.tensor_mul(a[:], attn_s[:], cx.adj_t[mt][:])
        A_t.append(a)
    return A_t


def _emit_theta_mms(cx, pq, b, nt, ch, kk, start, stop):
    """6 accumulating K=64 matmuls: psum[:, ti*64:+64] += T0_slice^T Theta_kk."""
    nc = cx.nc
    for ti in range(6):
        t = ch * 6 + ti
        nc.tensor.matmul(
            pq[:, ti * 64:(ti + 1) * 64],
            cx.T_t[(b, nt)][:, t * 128:(t + 1) * 128],
            cx.theta_t[:, kk * 64:(kk + 1) * 64],
            start=(start and ti == 0),
            stop=(stop and ti == 5),
        )


def _emit_w2(cx, b, W2):
    """W2 = X . Theta_2 in (t,o)-major layout, from host X^T tiles."""
    nc = cx.nc
    for nt in range(NT):
        pqs = []
        for ch in range(2):
            pq = cx.zp.tile([128, 384], F32, tag="zp", name="pq")
            _emit_theta_mms(cx, pq, b, nt, ch, 2, True, True)
            pqs.append(pq)
        for ch in range(2):
            nc.vector.tensor_copy(
                W2[:, nt * FT + ch * 384: nt * FT + (ch + 1) * 384], pqs[ch]
            )


def _emit_hop(cx, b, A_t, rhs_all, kk, dst):
    """dst = A^T rhs_all + X.Theta_kk  (16 psum groups; fused Theta MMs).

    If dst is None this is the output stage: relu-copy to out tiles + DMA.
    """
    nc = cx.nc
    for nt in range(NT):
        o_tile = None
        if dst is None:
            o_tile = cx.out_pool.tile([128, OT], F32, tag="out", name="o_tile")
        pzs = []
        for ch in range(2):
            pz = cx.zp.tile([128, 384], F32, tag="zp", name="pz")
            _emit_theta_mms(cx, pz, b, nt, ch, kk, True, False)
            pzs.append(pz)
        for ch in range(2):
            pz = pzs[ch]
            for mt in range(NT):
                nc.tensor.matmul(
                    pz[:],
                    A_t[mt][:, nt * 128:(nt + 1) * 128],
                    rhs_all[:, mt * FT + ch * 384: mt * FT + (ch + 1) * 384],
                    start=False,
                    stop=(mt == NT - 1),
                )
            if dst is not None:
                nc.vector.tensor_copy(
                    dst[:, nt * FT + ch * 384: nt * FT + (ch + 1) * 384], pz[:]
                )
            else:
                d = o_tile[:].rearrange("p (o t) -> p t o", t=T)[
                    :, ch * 6:(ch + 1) * 6, :
                ]
                s = pz[:].rearrange("p (t o) -> p t o", o=64)
                nc.scalar.activation(d, s, mybir.ActivationFunctionType.Relu)
        if dst is None:
            nc.sync.dma_start(
                cx.out_ap[b, nt * 128:(nt + 1) * 128, :, :].rearrange(
                    "p o t -> p (o t)"
                ),
                o_tile[:],
            )


def _build():
    nc = bacc.Bacc("TRN2", target_bir_lowering=False, debug=False)
    cx = Ctx()
    cx.nc = nc
    cx.xt_ap = nc.dram_tensor("xt", [BL, NT, F, T, 128], BF16, kind="ExternalInput").ap()
    cx.attn_ap = nc.dram_tensor(
        "spatial_attention", [BL, N, N], BF16, kind="ExternalInput"
    ).ap()
    cx.adj_ap = nc.dram_tensor("adj", [N, N], BF16, kind="ExternalInput").ap()
    cx.theta_ap = nc.dram_tensor("Theta", [K, F, O], BF16, kind="ExternalInput").ap()
    cx.out_ap = nc.dram_tensor("out", [BL, N, O, T], F32, kind="ExternalOutput").ap()

    with tile.TileContext(nc) as tc, ExitStack() as ctx:
        cx.a_pool = ctx.enter_context(tc.tile_pool(name="apool", bufs=2))
        cx.w_pool = ctx.enter_context(tc.tile_pool(name="wpool", bufs=2))
        cx.t_pool = ctx.enter_context(tc.tile_pool(name="tpool", bufs=1))
        cx.scr_pool = ctx.enter_context(tc.tile_pool(name="scr", bufs=3))
        cx.out_pool = ctx.enter_context(tc.tile_pool(name="outp", bufs=3))
        cx.const_pool = ctx.enter_context(tc.tile_pool(name="const", bufs=1))
        cx.zp = ctx.enter_context(tc.tile_pool(name="zp", bufs=7, space="PSUM"))

        cx.theta_t = cx.const_pool.tile([64, K * O], BF16, tag="theta")
        nc.sync.dma_start(cx.theta_t[:], cx.theta_ap.rearrange("k f o -> f k o"))
        cx.adj_t = []
        cx.attn_dep = None

        # T0 tiles: host-transposed X^T per (batch, n-tile): [64, T*128]
        cx.T_t = {}
        for b in range(BL):
            for nt in range(NT):
                cx.T_t[(b, nt)] = cx.t_pool.tile(
                    [64, T * 128], BF16, tag=f"T0_{nt}_{b}", name=f"T{b}_{nt}"
                )
        for b in range(BL):
            for nt in range(NT):
                cx.attn_dep = nc.sync.dma_start(
                    cx.T_t[(b, nt)][:],
                    cx.xt_ap[b, nt].rearrange("f t n -> f (t n)"),
                )

        # W2 for both batches early (PE filler during attn/adj DMA).
        W2 = [
            cx.w_pool.tile([128, NT * FT], BF16, tag="W2", name=f"W2_{b}")
            for b in range(BL)
        ]
        for b in range(BL):
            _emit_w2(cx, b, W2[b])

        for b in range(BL):
            A_t = _emit_A(cx, b, load_adj=(b == 0))
            if b == 0:
                cx.attn_dep = cx.last_attn  # gate batch-1 attn behind batch-0
            V = cx.w_pool.tile([128, NT * FT], BF16, tag="V", name=f"V_{b}")
            _emit_hop(cx, b, A_t, W2[b], 1, V)
            _emit_hop(cx, b, A_t, V, 0, None)

    nc.compile()
    return nc


def _make_in_maps(inputs):
    import ml_dtypes

    bf = ml_dtypes.bfloat16
    x = np.asarray(inputs["x"], dtype=np.float32).astype(bf)
    # [B, N, F, T] -> [B, NT, F, T, 128] so each (batch, n-tile) block is
    # one contiguous 192KB DMA
    xt = np.ascontiguousarray(
        x.reshape(B, NT, 128, F, T).transpose(0, 1, 3, 4, 2)
    )
    attn = np.ascontiguousarray(
        np.asarray(inputs["spatial_attention"], dtype=np.float32).astype(bf)
    )
    adj = np.ascontiguousarray(np.asarray(inputs["adj"], dtype=np.float32).astype(bf))
    theta = np.ascontiguousarray(
        np.asarray(inputs["Theta"], dtype=np.float32).astype(bf)
    )

    in_maps = []
    for i in range(N_CORES):
        s = slice(i * BL, (i + 1) * BL)
        in_maps.append(
            {
                "xt": xt[s],
                "spatial_attention": attn[s],
                "adj": adj,
                "Theta": theta,
            }
        )
    return in_maps


def kernel(**inputs):
    global _NC
    if _NC is None:
        _NC = _build()
    nc = _NC
    in_maps = _make_in_maps(inputs)
    res = run_bass_kernel_spmd(nc, in_maps, core_ids=list(range(N_CORES)))
    out = np.concatenate([res.results[i]["out"] for i in range(N_CORES)], axis=0)
    return out
